# revision 31
# baseline (speedup 1.0000x reference)
"""Trainium2 Bass kernel for a causal single-head attention block.

Reference computation (per batch b):
    q = x @ Wq + bq ; k = x @ Wk + bk ; v = x @ Wv + bv      (x: [S, D])
    logits = q @ k.T  (causal masked), probs = softmax(logits / sqrt(128))
    out = concat([x, probs @ v], axis=-1)                     -> [S, D+128]

Shapes are hardcoded: B=4, S=2048, D=1024, feature size 128, 8 NeuronCores.

Sharding (SPMD, one compiled graph for all 8 cores):
  core c -> batch b = c//2, interleave parity h = c%2.
  Each core computes the 8 query blocks (128 rows each) at global block
  positions {2j + h : j in 0..7} of its batch, and the K/V projection over
  the full 2048-row sequence of that batch.

  Local block layout is region-major: "even region" e_i = global block
  2i+h (the core's own query rows, loaded first), "odd region" o_i =
  global block 2i+(1-h).  Causality in local coords is then identical on
  every core: query block j attends to e_i for i<j fully, e_j with a
  triangular mask, o_i for i<j fully, and o_j either fully masked (h=0)
  or fully valid (h=1).

On-chip scheme (all matmuls bf16, fp32 PSUM accumulation):
  - host passes x^T as bf16, even region first, chunk-major -> Q/K
    projections start as soon as the first 512KB chunk lands and exactly
    match the DMA stream rate
  - kT, qT, vT = W.T @ x^T in matmul layout; evacuated by the Vector
    engine (bias add + bf16 cast) so the Scalar engine only runs exp
  - logits computed transposed, tiled to the arrival of qT chunks so exp
    starts as early as possible; causal masking is done by multiplying
    the diagonal-block exp by a 0/1 mask on the GpSimd engine (off the
    Scalar/Vector critical paths)
  - v natural layout via paired PE transposes of vT (bv folded in during
    the vT evacuation); ones column in vaug accumulates the softmax
    denominators in the PV accumulation group
  - normalize with reciprocal * per-partition scale, write fp32; the
    attention read streams out per query block on the sync ring
  - the x passthrough half of the output is the input tensor verbatim
    (pure data marshalling), assembled in the host-side unshard step
"""

import math

import numpy as np
import ml_dtypes

import concourse.bass as bass
import concourse.tile as tile
from concourse import bacc, mybir
from concourse.bass_utils import run_bass_kernel_spmd
from concourse.masks import make_identity

N_CORES = 8
B = 4
S = 2048  # sequence length per batch
D = 1024  # model dim
F = 128  # q/k/v feature size
NQ = 8  # query blocks of 128 rows per core
QROWS = NQ * 128  # 1024 local query rows per core
SCALE = 1.0 / math.sqrt(F)

FP32 = mybir.dt.float32
BF16 = mybir.dt.bfloat16
BF16_NP = ml_dtypes.bfloat16

_compiled = {}

# xT DMA chunks (columns within a region): finest first so the first
# projection matmuls start as early as possible.  Region 0 = even (own
# query rows), region 1 = odd.  Host lays xT out chunk-major.
EC = ((0, 256), (256, 256), (512, 512))
OC = ((0, 512), (512, 512))
CHUNKS = tuple((0, off, w) for off, w in EC) + tuple((1, off, w) for off, w in OC)


def _build():
    nc = bacc.Bacc("TRN2", target_bir_lowering=False, debug=False, num_devices=N_CORES)

    xT_ext = nc.dram_tensor("xT", [D * S], BF16, kind="ExternalInput")
    wq_ext = nc.dram_tensor("wq", [128, 8, 128], BF16, kind="ExternalInput")
    wk_ext = nc.dram_tensor("wk", [128, 8, 128], BF16, kind="ExternalInput")
    wv_ext = nc.dram_tensor("wv", [128, 8, 128], BF16, kind="ExternalInput")
    bq_ext = nc.dram_tensor("bq", [128, 1], FP32, kind="ExternalInput")
    bk_ext = nc.dram_tensor("bk", [128, 1], FP32, kind="ExternalInput")
    bv_ext = nc.dram_tensor("bv", [128, 1], FP32, kind="ExternalInput")
    pb_ext = nc.dram_tensor("pbias", [128, 1], FP32, kind="ExternalInput")
    mask_ext = nc.dram_tensor("masks", [128, 2, 128], BF16, kind="ExternalInput")
    read_ext = nc.dram_tensor("out_read", [QROWS, F], FP32, kind="ExternalOutput")

    with tile.TileContext(nc) as tc:
        with (
            tc.tile_pool(name="persist", bufs=1) as P,
            tc.tile_pool(name="ps_proj", bufs=2, space="PSUM") as ps_proj,
            tc.tile_pool(name="ps_tp", bufs=1, space="PSUM") as ps_tp,
            tc.tile_pool(name="ps_log", bufs=3, space="PSUM") as ps_log,
            tc.tile_pool(name="ps_read", bufs=2, space="PSUM") as ps_read,
        ):
            # ---- persistent SBUF tiles ----
            xT_sb = P.tile([128, 8, S], BF16)  # [d%128, d//128, rcol] r-major
            wq_sb = P.tile([128, 8, 128], BF16)
            wk_sb = P.tile([128, 8, 128], BF16)
            wv_sb = P.tile([128, 8, 128], BF16)
            bq_sb = P.tile([128, 1], FP32)
            bk_sb = P.tile([128, 1], FP32)
            bv_sb = P.tile([128, 1], FP32)
            pb_sb = P.tile([128, 1], FP32)  # odd-diag exp bias: -60 (h=0) / 0
            mask_sb = P.tile([128, 2, 128], BF16)  # 0/1 multiplicative
            ident = P.tile([128, 128], BF16)
            zero_sb = P.tile([128, 1], FP32)
            kT_sb = P.tile([128, 2, 8, 128], BF16)  # [feat, region, blk, s%128]
            qT_sb = P.tile([128, QROWS], BF16)  # [feat, local q]
            vT_sb = P.tile([128, S], BF16)  # [feat, rcol] region-major
            vaug_sb = P.tile([128, 2, 8, 132], BF16)  # [s%128, region, blk, v|1]
            expT_sb = P.tile([128, 2, 8, QROWS], BF16)  # [s%128, region, blk, q]
            read_sb = P.tile([128, NQ, 128], FP32)
            recip_sb = P.tile([128, NQ, 1], FP32)

            # ---- input DMAs.  Sync HWDGE ring: the xT stream (the sync
            # engine has nothing else to do, so blocking it is free).
            # Scalar HWDGE ring: weights/biases/masks, all early -- the
            # scalar engine must be DMA-free once exp work starts. ----
            xt_dmas = []
            base = 0
            for r, off, w in CHUNKS:
                n = 128 * 8 * w
                src = xT_ext[base:base + n].rearrange("(p t w) -> p t w", p=128, t=8)
                base += n
                dst = xT_sb[:, :, r * 1024 + off: r * 1024 + off + w]
                xt_dmas.append(nc.sync.dma_start(dst, src))

            nc.scalar.dma_start(wq_sb[:], wq_ext[:])
            nc.scalar.dma_start(wk_sb[:], wk_ext[:])
            nc.scalar.dma_start(bq_sb[:], bq_ext[:])
            nc.scalar.dma_start(bk_sb[:], bk_ext[:])
            nc.scalar.dma_start(wv_sb[:], wv_ext[:])
            nc.scalar.dma_start(bv_sb[:], bv_ext[:])
            nc.scalar.dma_start(pb_sb[:], pb_ext[:])
            nc.scalar.dma_start(mask_sb[:], mask_ext[:])

            nc.vector.memset(zero_sb[:], 0.0)
            nc.gpsimd.memset(vaug_sb[:, :, :, 128:129], 1.0)
            make_identity(nc, ident[:])

            # PE warm-up: the PE sits idle ~11us (preamble + first DMA), so
            # HAM throttles its clock to K=4/8 and the first real matmuls run
            # at half rate.  Keep it busy on dummy data during the DMA wait
            # so the real stream starts at the full 2.4 GHz.
            dummy_sb = P.tile([128, 512], BF16)
            nc.vector.memset(dummy_sb[:], 0.0)
            for _ in range(20):
                wu = ps_log.tile([128, 512], FP32, tag="log")
                nc.tensor.matmul(
                    wu[:], dummy_sb[:, 0:128], dummy_sb[:], start=True, stop=True
                )

            kT_flat = kT_sb[:].rearrange("p r b s -> p (r b s)")

            def proj(w_sb, b_sb, dst_flat, r, off, w):
                pp = ps_proj.tile([128, 512], FP32, tag="proj")
                for t in range(8):
                    nc.tensor.matmul(
                        pp[:, 0:w],
                        w_sb[:, t, :],
                        xT_sb[:, t, r * 1024 + off: r * 1024 + off + w],
                        start=(t == 0),
                        stop=(t == 7),
                    )
                nc.vector.tensor_scalar_add(
                    dst_flat[:, r * 1024 + off: r * 1024 + off + w],
                    pp[:, 0:w],
                    b_sb[:],
                )

            # logits tile: stationary kT[r, i], moving qT columns [q0, q1)
            def logits_tile(r, i, q0, q1):
                pl = ps_log.tile([128, 512], FP32, tag="log")
                w = q1 - q0
                nc.tensor.matmul(
                    pl[:, 0:w], kT_sb[:, r, i, :], qT_sb[:, q0:q1],
                    start=True, stop=True,
                )
                if r == 1 and q0 == i * 128:
                    # odd-region diagonal is all-masked (h=0) or all-valid
                    # (h=1): fold it into the exp bias (exp(x*s - 60) ~= 0)
                    # so no GpSimd hop sits on the PV critical chain
                    nc.scalar.activation(
                        expT_sb[:, r, i, q0:q0 + 128], pl[:, 0:128],
                        mybir.ActivationFunctionType.Exp, bias=pb_sb[:],
                        scale=SCALE,
                    )
                    if q1 > q0 + 128:
                        nc.scalar.activation(
                            expT_sb[:, r, i, q0 + 128:q1], pl[:, 128:w],
                            mybir.ActivationFunctionType.Exp, bias=zero_sb[:],
                            scale=SCALE,
                        )
                    return
                nc.scalar.activation(
                    expT_sb[:, r, i, q0:q1], pl[:, 0:w],
                    mybir.ActivationFunctionType.Exp, bias=zero_sb[:],
                    scale=SCALE,
                )
                if q0 == i * 128:
                    # even-region diagonal: triangular 0/1 mask multiply
                    # (GpSimd, SBUF-only; runs early, off the tail chain)
                    d = expT_sb[:, r, i, q0:q0 + 128]
                    nc.gpsimd.tensor_mul(d, d, mask_sb[:, r, :])

            # v natural layout: paired PE transposes of vT + one copy
            def v_nat(r, i):
                pt = ps_tp.tile([128, 256], BF16, tag="tp")
                for k in range(2):
                    nc.tensor.transpose(
                        pt[:, k * 128:(k + 1) * 128],
                        vT_sb[:, r * 1024 + (i + k) * 128: r * 1024 + (i + k + 1) * 128],
                        ident[:],
                    )
                nc.vector.tensor_copy(
                    vaug_sb[:, r, i:i + 2, 0:128],
                    pt[:].rearrange("p (b f) -> p b f", b=2),
                )

            # ---- even region: Q+K per chunk (matches DMA rate), logits
            # tiles as soon as their qT/kT deps can be met ----
            proj(wq_sb, bq_sb, qT_sb, 0, 0, 256)
            proj(wk_sb, bk_sb, kT_flat, 0, 0, 256)
            logits_tile(0, 0, 0, 256)
            logits_tile(0, 1, 128, 256)

            proj(wq_sb, bq_sb, qT_sb, 0, 256, 256)
            proj(wk_sb, bk_sb, kT_flat, 0, 256, 256)
            for i in range(2):
                logits_tile(0, i, 256, 512)
            logits_tile(0, 2, 256, 512)
            logits_tile(0, 3, 384, 512)

            proj(wq_sb, bq_sb, qT_sb, 0, 512, 512)
            proj(wk_sb, bk_sb, kT_flat, 0, 512, 512)
            for i in range(4):
                logits_tile(0, i, 512, 1024)
            for i in range(4, 8):
                logits_tile(0, i, i * 128, 1024)

            # even-region V + transposes (feed PV only, so emitted late)
            for off, w in ((0, 512), (512, 512)):
                proj(wv_sb, bv_sb, vT_sb, 0, off, w)
            for i in range(0, 8, 2):
                v_nat(0, i)

            # ---- odd region + PV, pipelined so the dependency tail after
            # the last xT chunk is as short as possible ----
            out_read = read_ext[:].rearrange("(g p) c -> p g c", p=128)

            def pv(j):
                pr = ps_read.tile([128, 129], FP32, tag="read")
                nmm = 2 * (j + 1)
                n = 0
                for i in range(j + 1):
                    for r in range(2):
                        nc.tensor.matmul(
                            pr[:],
                            expT_sb[:, r, i, j * 128:(j + 1) * 128],
                            vaug_sb[:, r, i, 0:129],
                            start=(n == 0),
                            stop=(n == nmm - 1),
                        )
                        n += 1
                nc.vector.reciprocal(recip_sb[:, j, :], pr[:, 128:129])
                nc.vector.tensor_scalar_mul(
                    read_sb[:, j, :], pr[:, 0:128], recip_sb[:, j, :]
                )
                nc.sync.dma_start(out=out_read[:, j, :], in_=read_sb[:, j, :])

            def lt_o(i):
                q0 = i * 128
                if q0 < 512:
                    logits_tile(1, i, q0, 512)
                    logits_tile(1, i, 512, 1024)
                else:
                    logits_tile(1, i, q0, 1024)

            proj(wk_sb, bk_sb, kT_flat, 1, 0, 512)
            for i in range(4):
                lt_o(i)
            proj(wv_sb, bv_sb, vT_sb, 1, 0, 512)
            v_nat(1, 0)
            v_nat(1, 2)

            proj(wk_sb, bk_sb, kT_flat, 1, 512, 512)
            for i in range(4, 8):
                lt_o(i)
            proj(wv_sb, bv_sb, vT_sb, 1, 512, 512)
            v_nat(1, 4)
            v_nat(1, 6)

            for j in range(NQ):
                pv(j)

    nc.compile()
    return nc


def _get_compiled():
    if "nc" not in _compiled:
        _compiled["nc"] = _build()
    return _compiled["nc"]


def _make_in_maps(inputs, Wq, bq, Wk, bk, Wv, bv):
    x = np.asarray(inputs, dtype=np.float32)
    assert x.shape == (B, S, D)

    def prep_w(w):
        w = np.asarray(w, dtype=np.float32).astype(BF16_NP)
        return np.ascontiguousarray(w.reshape(8, 128, 128).transpose(1, 0, 2))

    wq_np, wk_np, wv_np = prep_w(Wq), prep_w(Wk), prep_w(Wv)
    bq_np = np.asarray(bq, np.float32).reshape(128, 1)
    bk_np = np.asarray(bk, np.float32).reshape(128, 1)
    bv_np = np.asarray(bv, np.float32).reshape(128, 1)

    # 0/1 multiplicative masks [k, slot, q]: slot 0 = even-region diagonal
    # (lower-triangular in (q >= k)), slot 1 = odd-region diagonal (zero
    # for h=0, one for h=1)
    kk = np.arange(128)[:, None]
    qq = np.arange(128)[None, :]
    tri = (qq >= kk).astype(np.float32)
    m_h = []
    for h in range(2):
        other = np.full((128, 128), float(h), np.float32)
        m = np.stack([tri, other], axis=1)  # [k, slot, q]
        m_h.append(np.ascontiguousarray(m.astype(BF16_NP)))

    pb_h = [
        np.full((128, 1), -60.0, np.float32),
        np.zeros((128, 1), np.float32),
    ]
    in_maps = []
    for c in range(N_CORES):
        b, h = divmod(c, 2)
        xb = x[b].reshape(16, 128, D)  # [global block, row, D]
        pieces = []
        for r in range(2):
            par = h if r == 0 else 1 - h
            reg = xb[par::2].reshape(QROWS, D)  # [1024, D]
            regT = reg.T.astype(BF16_NP)  # [D, 1024]
            pieces.append(regT.reshape(8, 128, QROWS).transpose(1, 0, 2))
        # chunk-major flat, each chunk contiguous [p, t, w]
        xT = np.concatenate(
            [
                pieces[r][:, :, off:off + w].reshape(-1)
                for r, off, w in CHUNKS
            ]
        )
        in_maps.append(
            {
                "xT": xT,
                "wq": wq_np,
                "wk": wk_np,
                "wv": wv_np,
                "bq": bq_np,
                "bk": bk_np,
                "bv": bv_np,
                "masks": m_h[h],
                "pbias": pb_h[h],
            }
        )
    return in_maps


def _gather(inputs, results):
    # out = concat([x, read], -1): the x half is an input passthrough
    # (pure data marshalling), assembled here in the unshard step; the
    # device computes the attention read.
    out = np.empty((B, S, D + F), dtype=np.float32)
    out[:, :, :D] = np.asarray(inputs, dtype=np.float32)
    for c in range(N_CORES):
        b, h = divmod(c, 2)
        ord_ = results[c]["out_read"].reshape(NQ, 128, F)
        for j in range(NQ):
            g = 2 * j + h
            out[b, g * 128:(g + 1) * 128, D:] = ord_[j]
    return out


def run(inputs, Wq, bq, Wk, bk, Wv, bv, trace=False):
    """Build (cached), run on 8 cores, gather. Returns (output, results)."""
    nc = _get_compiled()
    in_maps = _make_in_maps(inputs, Wq, bq, Wk, bk, Wv, bv)
    if trace:
        try:
            res = run_bass_kernel_spmd(nc, in_maps, list(range(N_CORES)), trace=True)
            return _gather(inputs, res.results), res
        except Exception as e:  # profiling hook unavailable etc.
            print(f"trace run failed ({e!r}); falling back to untraced run")
    res = run_bass_kernel_spmd(nc, in_maps, list(range(N_CORES)))
    return _gather(inputs, res.results), res


def kernel(inputs, Wq, bq, Wk, bk, Wv, bv):
    out, _ = run(inputs, Wq, bq, Wk, bk, Wv, bv, trace=False)
    return out
